# revision 2
# baseline (speedup 1.0000x reference)
"""Trainium2 Bass kernel for nn_MultiHeadAttention_24421184045057.

Full-input contract: kernel(**inputs) takes the complete (unsharded) numpy
inputs of reference.setup_inputs() and returns (out, attn) matching
reference.reference(**inputs).

Sharding (8 NeuronCores): data-parallel over batch (2) x tensor-parallel over
head groups of 4 heads (4).  Each core projects q/k/v for its batch with the
column slice of wq/wk/wv for its heads, runs attention for its 4 heads, and
computes a partial output projection with its row-slice of wo.  The host sums
the 4 partials per batch (the "all-reduce") and assembles the attention
probabilities.

Device-side layout notes:
 - scores are computed transposed (scoresT[j, i] = scores[i, j]) so the
   key position j sits on the PSUM partition dim and softmax runs along it
   via the matmul itself: vh is augmented with a ones column, so the ctx
   matmul's PSUM row 64 accumulates sum_j exp(scores[i, j]) for free.
 - exp(scoresT) is stored UNNORMALIZED (bf16); the host divides by the
   returned denominators while transposing blocks into the final layout.
 - causal masking: fully-masked (j > i) blocks are skipped entirely (the host
   leaves zeros there, matching exp(-1e9) == 0 underflow); diagonal-crossing
   blocks get a -8e9 bias pattern added on the vector engine before exp.
"""

import sys

import numpy as np

if "/opt/trn_rl_repo" not in sys.path:
    sys.path.insert(0, "/opt/trn_rl_repo")

import ml_dtypes  # noqa: E402

BF16NP = ml_dtypes.bfloat16

B, S, D, H = 2, 2048, 1024, 16
DH = D // H            # 64
NCORES = 8
GH = 4                 # heads per core
GD = GH * DH           # 256 projected dims per core
NMT = GD // 128        # m tiles for q/k projections (2)
KT = D // 128          # k tiles over the model dim (8)
NST = S // 128         # 16 sequence tiles of 128
NCH = S // 512         # 4 sequence chunks of 512
SCALE = 0.125          # 1/sqrt(DH)
MASK_BIAS = np.float32(-8.0e9)   # scores + pat, then exp((scores+pat)/8)

_CACHE = {}


def _blocks(causal):
    """(h, c, a) triples in emission order; a = key j-tile, c = query chunk."""
    out = []
    for h in range(GH):
        for c in range(NCH):
            n_a = 4 * c + 4 if causal else 16
            for a in range(n_a):
                out.append((h, c, a))
    return out


def _build(causal):
    from concourse import bacc
    import concourse.mybir as mybir
    import concourse.tile as tile

    dt = mybir.dt
    F32, F32R, BF16 = dt.float32, dt.float32r, dt.bfloat16
    ALU = mybir.AluOpType
    AF = mybir.ActivationFunctionType

    nblk = len(_blocks(causal))

    nc = bacc.Bacc("TRN2", target_bir_lowering=False, debug=False,
                   num_devices=NCORES)

    qT = nc.dram_tensor("qT", [D, S], F32R, kind="ExternalInput")
    kT = nc.dram_tensor("kT", [D, S], F32R, kind="ExternalInput")
    vT_aug = nc.dram_tensor("vT_aug", [D + 1, S], BF16, kind="ExternalInput")
    wq = nc.dram_tensor("wq", [D, GD], F32R, kind="ExternalInput")
    wk = nc.dram_tensor("wk", [D, GD], F32R, kind="ExternalInput")
    wv_aug = nc.dram_tensor("wv_aug", [D + 1, GD], BF16, kind="ExternalInput")
    bq = nc.dram_tensor("bq", [128, NMT], F32, kind="ExternalInput")
    bk = nc.dram_tensor("bk", [128, NMT], F32, kind="ExternalInput")
    wo = nc.dram_tensor("wo", [GD, D], BF16, kind="ExternalInput")
    if causal:
        patterns = nc.dram_tensor("patterns", [4, 128, 512], F32,
                                  kind="ExternalInput")
    else:
        maskTb = nc.dram_tensor("maskTb", [S, S], F32, kind="ExternalInput")

    attn_out = nc.dram_tensor("attn_out", [nblk, 128, 512], BF16,
                              kind="ExternalOutput")
    den_out = nc.dram_tensor("den_out", [GH, S], F32, kind="ExternalOutput")
    out_partial = nc.dram_tensor("out_partial", [S, D], F32,
                                 kind="ExternalOutput")

    with tile.TileContext(nc) as tc:
        with tc.tile_pool(name="res", bufs=1) as res, \
             tc.tile_pool(name="work", bufs=2) as work:
            # ---- resident tiles -------------------------------------------
            qhT_t = [res.tile([128, S], F32R, name=f"qhT{m}") for m in range(NMT)]
            khT_t = [res.tile([128, S], F32R, name=f"khT{m}") for m in range(NMT)]
            vh_t = [res.tile([128, GH * (DH + 1)], BF16, name=f"vh{st}")
                    for st in range(NST)]
            ctxT_t = [res.tile([DH, S], BF16, name=f"ctxT{h}") for h in range(GH)]

            wq_t = [res.tile([128, GD], F32R, name=f"wq{kt}") for kt in range(KT)]
            wk_t = [res.tile([128, GD], F32R, name=f"wk{kt}") for kt in range(KT)]
            wv_t = [res.tile([128, GD], BF16, name=f"wv{kt}") for kt in range(KT)]
            wv_aug_t = res.tile([1, GD], BF16)
            bq_t = res.tile([128, NMT], F32)
            bk_t = res.tile([128, NMT], F32)
            wo_t = [res.tile([DH, D], BF16, name=f"wo{h}") for h in range(GH)]
            for kt in range(KT):
                nc.sync.dma_start(wq_t[kt][:], wq[128 * kt:128 * kt + 128, :])
                nc.sync.dma_start(wk_t[kt][:], wk[128 * kt:128 * kt + 128, :])
                nc.sync.dma_start(wv_t[kt][:], wv_aug[128 * kt:128 * kt + 128, :])
            nc.sync.dma_start(wv_aug_t[:], wv_aug[D:D + 1, :])
            nc.sync.dma_start(bq_t[:], bq[:])
            nc.sync.dma_start(bk_t[:], bk[:])
            for h in range(GH):
                nc.sync.dma_start(wo_t[h][:], wo[DH * h:DH * h + DH, :])
            if causal:
                pat_t = [res.tile([128, 512], F32, name=f"pat{p}")
                         for p in range(4)]
                for p in range(4):
                    nc.sync.dma_start(pat_t[p][:], patterns[p])

            # ---- q/k projections ------------------------------------------
            # qhT[m*128+p, s] = sum_D wq[D, m*128+p] * q[s, D] + bq
            for (xT_d, w_t, b_t, xhT_t) in ((qT, wq_t, bq_t, qhT_t),
                                            (kT, wk_t, bk_t, khT_t)):
                with tc.tile_pool(name="xt", bufs=KT) as xtp, \
                     tc.tile_pool(name="psp", bufs=3, space="PSUM") as psp:
                    xt = [xtp.tile([128, S], F32R, tag="xt", name=f"xt{kt}")
                          for kt in range(KT)]
                    for kt in range(KT):
                        nc.sync.dma_start(xt[kt][:],
                                          xT_d[128 * kt:128 * kt + 128, :])
                    for m in range(NMT):
                        for n in range(NCH):
                            ps = psp.tile([128, 512], F32, tag="proj")
                            for kt in range(KT):
                                nc.tensor.matmul(
                                    ps[:],
                                    w_t[kt][:, 128 * m:128 * m + 128],
                                    xt[kt][:, 512 * n:512 * n + 512],
                                    start=(kt == 0), stop=(kt == KT - 1))
                            nc.scalar.activation(
                                xhT_t[m][:, 512 * n:512 * n + 512], ps[:],
                                AF.Identity, bias=b_t[:, m:m + 1])

            # ---- v projection (natural layout, bias folded via aug row) ----
            with tc.tile_pool(name="vt", bufs=KT + 1) as vtp, \
                 tc.tile_pool(name="psv", bufs=3, space="PSUM") as psv:
                vt = [vtp.tile([128, S], BF16, tag="vt", name=f"vt{kt}")
                      for kt in range(KT)]
                vt_aug = vtp.tile([1, S], BF16)
                for kt in range(KT):
                    nc.sync.dma_start(vt[kt][:],
                                      vT_aug[128 * kt:128 * kt + 128, :])
                nc.sync.dma_start(vt_aug[:], vT_aug[D:D + 1, :])
                for st in range(NST):
                    ps = psv.tile([128, GD], F32, tag="projv")
                    for kt in range(KT):
                        nc.tensor.matmul(
                            ps[:], vt[kt][:, 128 * st:128 * st + 128],
                            wv_t[kt][:], start=(kt == 0), stop=False)
                    nc.tensor.matmul(
                        ps[:], vt_aug[:, 128 * st:128 * st + 128],
                        wv_aug_t[:], start=False, stop=True)
                    dst = vh_t[st][:].rearrange("p (h d) -> p h d", h=GH)
                    nc.scalar.copy(dst[:, :, 0:DH],
                                   ps[:].rearrange("p (h d) -> p h d", h=GH))
                    nc.vector.memset(dst[:, :, DH:DH + 1], 1.0)

            # ---- attention -------------------------------------------------
            blk = 0
            with tc.tile_pool(name="psa", bufs=1, space="PSUM") as psa, \
                 tc.tile_pool(name="exp", bufs=4) as expp, \
                 tc.tile_pool(name="gp", bufs=4) as gpp:
                for h in range(GH):
                    hp = 64 * (h % 2)      # partition offset inside qhT/khT tile
                    mt = h // 2
                    for c in range(NCH):
                        n_a = 4 * c + 4 if causal else 16
                        ctx = psa.tile([DH + 1, 512], F32, tag="ctx", bufs=2)
                        for a0 in range(0, n_a, 2):
                            sc = psa.tile([128, 1024], F32, tag="sc", bufs=3)
                            for di in range(2):
                                a = a0 + di
                                nc.tensor.matmul(
                                    sc[:, 512 * di:512 * di + 512],
                                    khT_t[mt][hp:hp + DH, 128 * a:128 * a + 128],
                                    qhT_t[mt][hp:hp + DH, 512 * c:512 * c + 512],
                                    start=True, stop=True)
                            for di in range(2):
                                a = a0 + di
                                if causal:
                                    ragged = a >= 4 * c
                                    pat = pat_t[a % 4][:] if ragged else None
                                else:
                                    ragged = True
                                    gp = gpp.tile([128, 512], F32, tag="gpat")
                                    nc.sync.dma_start(
                                        gp[:],
                                        maskTb[128 * a:128 * a + 128,
                                               512 * c:512 * c + 512])
                                    pat = gp[:]
                                if ragged:
                                    sl = sc[:, 512 * di:512 * di + 512]
                                    nc.vector.scalar_tensor_tensor(
                                        sl, sl, 1.0, pat, ALU.mult, ALU.add)
                            ex = expp.tile([128, 1024], BF16, tag="ex")
                            nc.scalar.activation(ex[:], sc[:], AF.Exp,
                                                 scale=SCALE)
                            for di in range(2):
                                a = a0 + di
                                nc.tensor.matmul(
                                    ctx[:],
                                    vh_t[a][:, (DH + 1) * h:(DH + 1) * h + DH + 1],
                                    ex[:, 512 * di:512 * di + 512],
                                    start=(a == 0), stop=(a == n_a - 1))
                            nc.sync.dma_start(
                                attn_out[blk:blk + 2].rearrange("n p f -> p n f"),
                                ex[:].rearrange("p (n f) -> p n f", n=2))
                            blk += 2
                        # denominator -> host + ctxT normalization
                        den0 = work.tile([1, 512], F32, tag="den")
                        nc.scalar.copy(den0[:], ctx[DH:DH + 1, :])
                        nc.sync.dma_start(
                            den_out[h:h + 1, 512 * c:512 * c + 512], den0[:])
                        rc = work.tile([1, 512], F32, tag="rc")
                        nc.vector.reciprocal(rc[:], den0[:])
                        bt = work.tile([DH, 512], F32, tag="bt")
                        nc.gpsimd.partition_broadcast(bt[:], rc[:])
                        nc.vector.tensor_tensor(
                            ctxT_t[h][:, 512 * c:512 * c + 512],
                            ctx[0:DH, :], bt[:], ALU.mult)

            # ---- output projection ----------------------------------------
            with tc.tile_pool(name="pso", bufs=4, space="PSUM") as pso, \
                 tc.tile_pool(name="outp", bufs=3) as outp:
                for st in range(NST):
                    ot = outp.tile([128, D], F32, tag="ot")
                    for n in range(2):
                        ps = pso.tile([128, 512], F32, tag="o")
                        for hh in range(GH):
                            nc.tensor.matmul(
                                ps[:],
                                ctxT_t[hh][:, 128 * st:128 * st + 128],
                                wo_t[hh][:, 512 * n:512 * n + 512],
                                start=(hh == 0), stop=(hh == GH - 1))
                        nc.vector.tensor_copy(ot[:, 512 * n:512 * n + 512],
                                              ps[:])
                    nc.sync.dma_start(
                        out_partial[128 * st:128 * st + 128, :], ot[:])

    nc.compile()
    return nc


def _causal_patterns():
    """pat[p][j, i] = MASK_BIAS where (128*p + j) > i else 0, (128, 512)."""
    j = np.arange(128)[:, None]
    i = np.arange(512)[None, :]
    pats = np.zeros((4, 128, 512), np.float32)
    for p in range(4):
        pats[p] = np.where(128 * p + j > i, MASK_BIAS, np.float32(0.0))
    return pats


def _get_program(causal):
    if causal not in _CACHE:
        _CACHE[causal] = _build(causal)
    return _CACHE[causal]


def kernel(q, k, v, mask, wq, bq, wk, bk, wv, bv, wo, bo):
    from concourse.bass_utils import run_bass_kernel_spmd

    q = np.asarray(q, np.float32)
    k = np.asarray(k, np.float32)
    v = np.asarray(v, np.float32)
    mask2d = np.asarray(mask, np.float32).reshape(S, S)
    wq = np.asarray(wq, np.float32); bq = np.asarray(bq, np.float32)
    wk = np.asarray(wk, np.float32); bk = np.asarray(bk, np.float32)
    wv = np.asarray(wv, np.float32); bv = np.asarray(bv, np.float32)
    wo = np.asarray(wo, np.float32); bo = np.asarray(bo, np.float32)

    causal = bool(
        np.array_equal(mask2d, np.triu(np.ones((S, S), np.float32), k=1)))
    nc = _get_program(causal)

    # ---- per-batch / per-group host prep ---------------------------------
    ones_row = np.ones((1, S), np.float32)
    qT_b = [np.ascontiguousarray(q[b].T) for b in range(B)]
    kT_b = [np.ascontiguousarray(k[b].T) for b in range(B)]
    vTaug_b = [np.concatenate([v[b].T, ones_row], axis=0).astype(BF16NP)
               for b in range(B)]
    if causal:
        pats = _causal_patterns()
    else:
        maskTb = np.ascontiguousarray(mask2d.T) * MASK_BIAS

    in_maps = []
    for core in range(NCORES):
        b, g = divmod(core, 4)
        sl = slice(GD * g, GD * g + GD)
        m = {
            "qT": qT_b[b],
            "kT": kT_b[b],
            "vT_aug": vTaug_b[b],
            "wq": np.ascontiguousarray(wq[:, sl]),
            "wk": np.ascontiguousarray(wk[:, sl]),
            "wv_aug": np.concatenate(
                [wv[:, sl], bv[sl][None, :]], axis=0).astype(BF16NP),
            "bq": np.ascontiguousarray(bq[sl].reshape(NMT, 128).T),
            "bk": np.ascontiguousarray(bk[sl].reshape(NMT, 128).T),
            "wo": wo[sl, :].astype(BF16NP),
        }
        if causal:
            m["patterns"] = pats
        else:
            m["maskTb"] = maskTb
        in_maps.append(m)

    res = run_bass_kernel_spmd(nc, in_maps, list(range(NCORES)))

    # ---- host assembly ----------------------------------------------------
    out = np.zeros((B, S, D), np.float32)
    attn = np.zeros((B, H, S, S), np.float32)
    blocks = _blocks(causal)
    for core in range(NCORES):
        b, g = divmod(core, 4)
        r = res.results[core]
        out[b] += r["out_partial"]
        packed = np.asarray(r["attn_out"], np.float32)
        den = np.asarray(r["den_out"], np.float32)
        blk = 0
        for h in range(GH):
            for c in range(NCH):
                n_a = 4 * c + 4 if causal else 16
                a_blocks = packed[blk:blk + n_a]          # (n_a, 128, 512)
                blk += n_a
                bt = a_blocks.transpose(2, 0, 1).reshape(512, n_a * 128)
                attn[b, GH * g + h, 512 * c:512 * c + 512, 0:n_a * 128] = (
                    bt / den[h, 512 * c:512 * c + 512][:, None])
    out += bo[None, None, :]
    return out, attn


# revision 22
# speedup vs baseline: 1.1901x; 1.1901x over previous
"""Trainium2 Bass kernel for nn_MultiHeadAttention_24421184045057.

Full-input contract: kernel(**inputs) takes the complete (unsharded) numpy
inputs of reference.setup_inputs() and returns (out, attn) matching
reference.reference(**inputs).

Sharding (8 NeuronCores): data-parallel over batch (2) x tensor-parallel over
head groups of 4 heads (4).  Each core projects q/k/v for its batch with the
column slice of wq/wk/wv for its heads, runs attention for its 4 heads, and
computes a partial output projection with its row-slice of wo.  The host sums
the 4 partials per batch (the "all-reduce") and assembles the attention
probabilities.

Device-side layout notes:
 - scores are computed transposed (scoresT[j, i] = scores[i, j]) so the
   key position j sits on the PSUM partition dim and softmax runs along it
   via the matmul itself: vh is augmented with a ones column, so the ctx
   matmul's PSUM row 64 accumulates sum_j exp(scores[i, j]) for free.
 - exp(scoresT) is stored UNNORMALIZED (bf16); the host divides by the
   returned denominators while transposing blocks into the final layout.
 - causal masking: fully-masked (j > i) blocks are skipped entirely (the host
   leaves zeros there, matching exp(-1e9) == 0 underflow); diagonal-crossing
   blocks get a -8e9 bias pattern added on the vector engine before exp.
"""

import sys

import numpy as np

if "/opt/trn_rl_repo" not in sys.path:
    sys.path.insert(0, "/opt/trn_rl_repo")

import ml_dtypes  # noqa: E402

BF16NP = ml_dtypes.bfloat16

B, S, D, H = 2, 2048, 1024, 16
DH = D // H            # 64
NCORES = 8
GH = 4                 # heads per core
GD = GH * DH           # 256 projected dims per core
NMT = GD // 128        # m tiles for q/k projections (2)
KT = D // 128          # k tiles over the model dim (8)
NST = S // 128         # 16 sequence tiles of 128
NCH = S // 512         # 4 sequence chunks of 512
SCALE = 0.125          # 1/sqrt(DH)
MASK_BIAS = np.float32(-8.0e9)   # scores + pat, then exp((scores+pat)/8)

_CACHE = {}


def _blocks(causal):
    """(c, h, a) triples in emission order; a = key j-tile, c = query chunk."""
    out = []
    for c in range(NCH):
        for h in range(GH):
            n_a = 4 * c + 4 if causal else 16
            for a in range(n_a):
                out.append((c, h, a))
    return out


def _emit_outproj(nc, tc, psm, outp, ctxT_t, wo_t, out_partial, c):
    import concourse.mybir as mybir
    F32 = mybir.dt.float32
    for st in range(4 * c, 4 * c + 4):
        ot = outp.tile([128, D], F32, tag="ot")
        for n in range(2):
            ps = psm.tile([128, 512], F32, tag="proj", bufs=2)
            for hh in range(GH):
                nc.tensor.matmul(
                    ps[:], ctxT_t[hh][:, 128 * st:128 * st + 128],
                    wo_t[:, D * hh + 512 * n:D * hh + 512 * n + 512],
                    start=(hh == 0), stop=(hh == GH - 1))
            nc.vector.tensor_copy(ot[:, 512 * n:512 * n + 512], ps[:])
        nc.sync.dma_start(out_partial[128 * st:128 * st + 128, :], ot[:])


def _build(causal):
    from concourse import bacc
    import concourse.mybir as mybir
    import concourse.tile as tile

    dt = mybir.dt
    F32, F32R, BF16 = dt.float32, dt.float32r, dt.bfloat16
    ALU = mybir.AluOpType
    AF = mybir.ActivationFunctionType

    nblk = len(_blocks(causal))

    nc = bacc.Bacc("TRN2", target_bir_lowering=False, debug=False,
                   num_devices=NCORES)

    qT = nc.dram_tensor("qT", [D, S], F32R, kind="ExternalInput")
    kT = nc.dram_tensor("kT", [D, S], F32R, kind="ExternalInput")
    vT_aug = nc.dram_tensor("vT_aug", [D + 1, S], BF16, kind="ExternalInput")
    wq = nc.dram_tensor("wq", [D, GD], F32R, kind="ExternalInput")
    wk = nc.dram_tensor("wk", [D, GD], F32R, kind="ExternalInput")
    wv_aug = nc.dram_tensor("wv_aug", [D + 1, GD], BF16, kind="ExternalInput")
    bq = nc.dram_tensor("bq", [128, NMT], F32, kind="ExternalInput")
    bk = nc.dram_tensor("bk", [128, NMT], F32, kind="ExternalInput")
    wo = nc.dram_tensor("wo", [GD, D], BF16, kind="ExternalInput")
    if causal:
        patterns = nc.dram_tensor("patterns", [128, 2048], F32,
                                  kind="ExternalInput")
    else:
        maskTb = nc.dram_tensor("maskTb", [S, S], F32, kind="ExternalInput")

    attn_out = nc.dram_tensor("attn_out", [nblk, 128, 512], BF16,
                              kind="ExternalOutput")
    den_out = nc.dram_tensor("den_out", [GH, S], F32, kind="ExternalOutput")
    out_partial = nc.dram_tensor("out_partial", [S, D], F32,
                                 kind="ExternalOutput")

    with tile.TileContext(nc) as tc:
        with tc.tile_pool(name="res", bufs=1) as res, \
             tc.tile_pool(name="work", bufs=2) as work, \
             tc.tile_pool(name="psm", bufs=1, space="PSUM") as psm, \
             tc.tile_pool(name="exp", bufs=3) as expp, \
             tc.tile_pool(name="gp", bufs=4) as gpp, \
             tc.tile_pool(name="outp", bufs=3) as outp:
            # ---- resident tiles -------------------------------------------
            qhT_t = [res.tile([128, S], F32R, name=f"qhT{m}") for m in range(NMT)]
            khT_t = [res.tile([128, S], F32R, name=f"khT{m}") for m in range(NMT)]
            vh_t = [res.tile([128, GH * (DH + 1)], BF16, name=f"vh{st}")
                    for st in range(NST)]
            ctxT_t = [res.tile([DH, S], BF16, name=f"ctxT{h}") for h in range(GH)]

            wq_t = res.tile([128, KT * GD], F32R)   # [:, 256*kt+d]
            wk_t = res.tile([128, KT * GD], F32R)
            wv_t = res.tile([128, KT * GD], BF16)
            wv_aug_t = res.tile([1, GD], BF16)
            bq_t = res.tile([128, NMT], F32)
            bk_t = res.tile([128, NMT], F32)
            wo_t = res.tile([DH, GH * D], BF16)     # [:, 1024*h+e]
            nc.gpsimd.dma_start(bq_t[:], bq[:])
            nc.gpsimd.dma_start(bk_t[:], bk[:])
            if causal:
                pat_t = res.tile([128, 2048], F32)  # [:, 512*p+i]

            # ---- fully chunk-streamed pipeline ----------------------------
            # causal: query chunk c only needs k/v sequence tiles <= 4c+3, so
            # k-proj, v-proj, q-proj, attention, and the previous chunk's out
            # projection are all emitted per chunk and overlap.

            with tc.tile_pool(name="xt", bufs=3) as xtp:
                blk = 0
                for c in range(NCH):
                    cs = slice(512 * c, 512 * c + 512)
                    # -- k projection, chunk c --
                    if c == 0:
                        nc.sync.dma_start(
                            wk_t[:].rearrange("p (kt d) -> p kt d", kt=KT),
                            wk.rearrange("(kt p) d -> p kt d", p=128))
                    kx = xtp.tile([128, KT * 512], F32R, tag="xt")
                    nc.sync.dma_start(
                        kx[:].rearrange("p (kt s) -> p kt s", kt=KT),
                        kT.rearrange("(kt p) s -> p kt s", p=128)[:, :, cs])
                    for m in range(NMT):
                        ps = psm.tile([128, 512], F32, tag="proj", bufs=2)
                        for kt in range(KT):
                            nc.tensor.matmul(
                                ps[:],
                                wk_t[:, GD * kt + 128 * m:GD * kt + 128 * m + 128],
                                kx[:, 512 * kt:512 * kt + 512],
                                start=(kt == 0), stop=(kt == KT - 1))
                        nc.scalar.activation(khT_t[m][:, cs], ps[:],
                                             AF.Identity, bias=bk_t[:, m:m + 1])
                    # -- v projection, s-tiles of chunk c --
                    if c == 0:
                        nc.sync.dma_start(
                            wv_t[:].rearrange("p (kt d) -> p kt d", kt=KT),
                            wv_aug[0:D].rearrange("(kt p) d -> p kt d", p=128))
                        nc.gpsimd.dma_start(wv_aug_t[:], wv_aug[D:D + 1, :])
                    vx = xtp.tile([128, KT * 512], BF16, tag="vx", bufs=2)
                    nc.sync.dma_start(
                        vx[:].rearrange("p (kt s) -> p kt s", kt=KT),
                        vT_aug[0:D].rearrange("(kt p) s -> p kt s", p=128)[:, :, cs])
                    vxa = xtp.tile([1, 512], BF16, tag="vxa", bufs=2)
                    nc.gpsimd.dma_start(vxa[:], vT_aug[D:D + 1, cs])
                    for sl in range(4):
                        st = 4 * c + sl
                        ps = psm.tile([128, GD], F32, tag="proj", bufs=2)
                        for kt in range(KT):
                            nc.tensor.matmul(
                                ps[:],
                                vx[:, 512 * kt + 128 * sl:512 * kt + 128 * sl + 128],
                                wv_t[:, GD * kt:GD * kt + GD],
                                start=(kt == 0), stop=False)
                        nc.tensor.matmul(
                            ps[:], vxa[:, 128 * sl:128 * sl + 128],
                            wv_aug_t[:], start=False, stop=True)
                        dst = vh_t[st][:].rearrange("p (h d) -> p h d", h=GH)
                        nc.scalar.copy(dst[:, :, 0:DH],
                                       ps[:].rearrange("p (h d) -> p h d", h=GH))
                        nc.vector.memset(dst[:, :, DH:DH + 1], 1.0)
                    # -- q projection, chunk c --
                    if c == 0:
                        nc.sync.dma_start(
                            wq_t[:].rearrange("p (kt d) -> p kt d", kt=KT),
                            wq.rearrange("(kt p) d -> p kt d", p=128))
                    qx = xtp.tile([128, KT * 512], F32R, tag="xt")
                    nc.sync.dma_start(
                        qx[:].rearrange("p (kt s) -> p kt s", kt=KT),
                        qT.rearrange("(kt p) s -> p kt s", p=128)[:, :, cs])
                    if c == 0:
                        if causal:
                            nc.sync.dma_start(pat_t[:], patterns[:])
                        nc.sync.dma_start(
                            wo_t[:].rearrange("p (h e) -> p h e", h=GH),
                            wo.rearrange("(h p) e -> p h e", p=DH))
                    for m in range(NMT):
                        ps = psm.tile([128, 512], F32, tag="proj", bufs=2)
                        for kt in range(KT):
                            nc.tensor.matmul(
                                ps[:],
                                wq_t[:, GD * kt + 128 * m:GD * kt + 128 * m + 128],
                                qx[:, 512 * kt:512 * kt + 512],
                                start=(kt == 0), stop=(kt == KT - 1))
                        nc.scalar.activation(qhT_t[m][:, cs], ps[:],
                                             AF.Identity, bias=bq_t[:, m:m + 1])

                    # -- out projection for the previous chunk (PE filler) --
                    if c > 0:
                        _emit_outproj(nc, tc, psm, outp, ctxT_t, wo_t,
                                      out_partial, c - 1)

                    # -- attention, chunk c, all heads --
                    n_a = 4 * c + 4 if causal else 16
                    for h in range(GH):
                        hp = 64 * (h % 2)
                        mt = h // 2
                        ctx = psm.tile([DH + 1, 512], F32, tag="ctx", bufs=2)

                        def emit_scores(a0):
                            ex = expp.tile([128, 2048], BF16, tag="ex")
                            for half in range(2):
                                sc = psm.tile([128, 1024], F32, tag="sc",
                                              bufs=2)
                                for di in range(2):
                                    a = a0 + 2 * half + di
                                    nc.tensor.matmul(
                                        sc[:, 512 * di:512 * di + 512],
                                        khT_t[mt][hp:hp + DH,
                                                  128 * a:128 * a + 128],
                                        qhT_t[mt][hp:hp + DH, cs],
                                        start=True, stop=True)
                                for di in range(2):
                                    a = a0 + 2 * half + di
                                    if causal:
                                        ragged = a >= 4 * c
                                        pat = (pat_t[:, 512 * (a % 4):
                                                     512 * (a % 4) + 512]
                                               if ragged else None)
                                    else:
                                        ragged = True
                                        gp = gpp.tile([128, 512], F32,
                                                      tag="gpat")
                                        nc.sync.dma_start(
                                            gp[:],
                                            maskTb[128 * a:128 * a + 128, cs])
                                        pat = gp[:]
                                    if ragged:
                                        sl2 = sc[:, 512 * di:512 * di + 512]
                                        nc.vector.scalar_tensor_tensor(
                                            sl2, sl2, 1.0, pat, ALU.mult,
                                            ALU.add)
                                nc.scalar.activation(
                                    ex[:, 1024 * half:1024 * half + 1024],
                                    sc[:], AF.Exp, scale=SCALE)
                            return ex

                        def emit_ctx(a0, ex, blk0):
                            for qi in range(4):
                                a = a0 + qi
                                nc.tensor.matmul(
                                    ctx[:],
                                    vh_t[a][:, (DH + 1) * h:
                                            (DH + 1) * h + DH + 1],
                                    ex[:, 512 * qi:512 * qi + 512],
                                    start=(a == 0), stop=(a == n_a - 1))
                            nc.scalar.dma_start(
                                attn_out[blk0:blk0 + 4].rearrange(
                                    "n p f -> p n f"),
                                ex[:].rearrange("p (n f) -> p n f", n=4))

                        for a0 in range(0, n_a, 4):
                            ex = emit_scores(a0)
                            emit_ctx(a0, ex, blk)
                            blk += 4
                        # denominator -> host + ctxT normalization
                        den0 = work.tile([1, 512], F32, tag="den", bufs=3)
                        nc.vector.tensor_copy(den0[:], ctx[DH:DH + 1, :])
                        nc.gpsimd.dma_start(den_out[h:h + 1, cs], den0[:])
                        rc = work.tile([1, 512], F32, tag="rc", bufs=3)
                        nc.vector.reciprocal(rc[:], den0[:])
                        bt = work.tile([DH, 512], F32, tag="bt", bufs=3)
                        nc.gpsimd.partition_broadcast(bt[:], rc[:])
                        nc.vector.tensor_tensor(
                            ctxT_t[h][:, cs], ctx[0:DH, :], bt[:], ALU.mult)

                # tail: out projection of the last chunk
                _emit_outproj(nc, tc, psm, outp, ctxT_t, wo_t, out_partial,
                              NCH - 1)

    nc.compile()
    return nc


def _causal_patterns():
    """pat[p][j, i] = MASK_BIAS where (128*p + j) > i else 0, (128, 512)."""
    j = np.arange(128)[:, None]
    i = np.arange(512)[None, :]
    pats = np.zeros((4, 128, 512), np.float32)
    for p in range(4):
        pats[p] = np.where(128 * p + j > i, MASK_BIAS, np.float32(0.0))
    return pats


def _get_program(causal):
    if causal not in _CACHE:
        _CACHE[causal] = _build(causal)
    return _CACHE[causal]


def kernel(q, k, v, mask, wq, bq, wk, bk, wv, bv, wo, bo):
    from concourse.bass_utils import run_bass_kernel_spmd

    q = np.asarray(q, np.float32)
    k = np.asarray(k, np.float32)
    v = np.asarray(v, np.float32)
    mask2d = np.asarray(mask, np.float32).reshape(S, S)
    wq = np.asarray(wq, np.float32); bq = np.asarray(bq, np.float32)
    wk = np.asarray(wk, np.float32); bk = np.asarray(bk, np.float32)
    wv = np.asarray(wv, np.float32); bv = np.asarray(bv, np.float32)
    wo = np.asarray(wo, np.float32); bo = np.asarray(bo, np.float32)

    causal = bool(
        np.array_equal(mask2d, np.triu(np.ones((S, S), np.float32), k=1)))
    nc = _get_program(causal)

    # ---- per-batch / per-group host prep ---------------------------------
    ones_row = np.ones((1, S), np.float32)
    qT_b = [np.ascontiguousarray(q[b].T) for b in range(B)]
    kT_b = [np.ascontiguousarray(k[b].T) for b in range(B)]
    vTaug_b = [np.concatenate([v[b].T, ones_row], axis=0).astype(BF16NP)
               for b in range(B)]
    if causal:
        # device layout: (128, 4*512) with pattern p at columns [512p, 512p+512)
        pats = np.ascontiguousarray(
            _causal_patterns().transpose(1, 0, 2).reshape(128, 2048))
    else:
        maskTb = np.ascontiguousarray(mask2d.T) * MASK_BIAS

    in_maps = []
    for core in range(NCORES):
        b, g = divmod(core, 4)
        sl = slice(GD * g, GD * g + GD)
        m = {
            "qT": qT_b[b],
            "kT": kT_b[b],
            "vT_aug": vTaug_b[b],
            "wq": np.ascontiguousarray(wq[:, sl]),
            "wk": np.ascontiguousarray(wk[:, sl]),
            "wv_aug": np.concatenate(
                [wv[:, sl], bv[sl][None, :]], axis=0).astype(BF16NP),
            "bq": np.ascontiguousarray(bq[sl].reshape(NMT, 128).T),
            "bk": np.ascontiguousarray(bk[sl].reshape(NMT, 128).T),
            "wo": wo[sl, :].astype(BF16NP),
        }
        if causal:
            m["patterns"] = pats
        else:
            m["maskTb"] = maskTb
        in_maps.append(m)

    res = run_bass_kernel_spmd(nc, in_maps, list(range(NCORES)))

    # ---- host assembly ----------------------------------------------------
    out = np.zeros((B, S, D), np.float32)
    attn = np.zeros((B, H, S, S), np.float32)
    blocks = _blocks(causal)
    for core in range(NCORES):
        b, g = divmod(core, 4)
        r = res.results[core]
        out[b] += r["out_partial"]
        packed = np.asarray(r["attn_out"], np.float32)
        den = np.asarray(r["den_out"], np.float32)
        blk = 0
        for c in range(NCH):
            for h in range(GH):
                n_a = 4 * c + 4 if causal else 16
                a_blocks = packed[blk:blk + n_a]          # (n_a, 128, 512)
                blk += n_a
                bt = a_blocks.transpose(2, 0, 1).reshape(512, n_a * 128)
                attn[b, GH * g + h, 512 * c:512 * c + 512, 0:n_a * 128] = (
                    bt / den[h, 512 * c:512 * c + 512][:, None])
    out += bo[None, None, :]
    return out, attn
